# revision 1
# baseline (speedup 1.0000x reference)
"""DistMult edge-scoring kernel for Trainium2 (8 NeuronCores, SPMD).

score[j] = sum_d emb_A[a_idx[j], d] * k[d] * emb_B[b_idx[j], d]
for 9E pairs: E positive edges, 4E head-corrupted, 4E tail-corrupted.

Strategy (v3, hybrid dense/gather — exploits the repeat structure):
- The positive-edge rows and the repeated rows (b-side of head mode,
  a-side of tail mode, both k-prescaled on the host) are uploaded as
  DENSE per-pair arrays and streamed with plain HWDGE DMA.
- Only the corrupt heads/tails are gathered on-device via
  gpsimd.dma_gather (int16 chunk-local indices, tables split in 4
  chunks of 25000 rows, pairs sorted by chunk on the host). Gathers
  round-robin over 4 SWDGE queues (descriptor generation on the Q7
  cores is the bottleneck; 4 queues parallelize it).
- All 9E pairs are dealt round-robin across the 8 cores in 128-pair
  sub-slots so every core runs an identical instruction stream (true
  SPMD). The program is built after seeing the data; compile is cached
  on the group-slot signature.
- Compute: one fused scalar_tensor_tensor (mul + accumulate-reduce) per
  128-pair slot on the vector engine.
- Host inverse-permutes the scores back to reference order.
"""

import numpy as np

# problem constants
N_A = 100000
N_B = 100000
D = 128
E = 100000
NEG = 4
NCORES = 8

P = 128
CHUNK = 25000          # table rows per int16-indexable chunk
NCHUNKS = 4
BATCH_SLOTS = 8        # 128-pair slots per batch (num_idxs=1024 HW ceiling)
SUB = P * NCORES       # pairs per dealt slot-row (1024)

_CACHED = {}


def _build_program(pos_slots, head_slots, tail_slots):
    """head_slots/tail_slots: per-chunk slot counts (len 4). Same for all
    cores. Program: pos (dense+dense), head (gather-a + dense-b), tail
    (dense-a + gather-b)."""
    import concourse.tile as tile
    from concourse import bacc, mybir

    f32 = mybir.dt.float32
    i16 = mybir.dt.int16
    mult = mybir.AluOpType.mult

    nh = sum(head_slots)
    nt = sum(tail_slots)
    total_slots = pos_slots + nh + nt

    nc = bacc.Bacc("TRN2", target_bir_lowering=False, debug=False,
                   num_devices=NCORES, num_swdge_queues=4)
    embA = nc.dram_tensor("emb_a", [N_A, D], f32, kind="ExternalInput").ap()
    embB = nc.dram_tensor("emb_b", [N_B, D], f32, kind="ExternalInput").ap()
    pos_a_d = nc.dram_tensor("pos_a", [P, pos_slots * D], f32,
                             kind="ExternalInput").ap()
    pos_b_d = nc.dram_tensor("pos_b", [P, pos_slots * D], f32,
                             kind="ExternalInput").ap()
    hidx_d = nc.dram_tensor("head_idx", [P, nh * 8], i16,
                            kind="ExternalInput").ap()
    hdense_d = nc.dram_tensor("head_dense", [P, nh * D], f32,
                              kind="ExternalInput").ap()
    tidx_d = nc.dram_tensor("tail_idx", [P, nt * 8], i16,
                            kind="ExternalInput").ap()
    tdense_d = nc.dram_tensor("tail_dense", [P, nt * D], f32,
                              kind="ExternalInput").ap()
    s_out = nc.dram_tensor("scores", [P, total_slots], f32,
                           kind="ExternalOutput").ap()

    # (table_ap, chunk, idx dram, dense dram, idx col0, dense col0, n_slots)
    gather_batches = []

    def section_batches(slots_per_chunk, idx_d, dense_d, table):
        out = []
        col = 0
        for c, gs in enumerate(slots_per_chunk):
            left = gs
            while left > 0:
                n = min(left, BATCH_SLOTS)
                out.append((table, c, idx_d, dense_d, col, n))
                col += n
                left -= n
        return out

    hb = section_batches(head_slots, hidx_d, hdense_d, embA)
    tb = section_batches(tail_slots, tidx_d, tdense_d, embB)
    # interleave head/tail so both tables' gathers spread over queues
    gather_batches = [b for pair in
                      zip(hb + [None] * len(tb), tb + [None] * len(hb))
                      for b in pair if b is not None][:len(hb) + len(tb)]

    with tile.TileContext(nc) as tc:
        with (
            tc.tile_pool(name="idx", bufs=8) as idx_pool,
            tc.tile_pool(name="gather", bufs=8) as g_pool,
            tc.tile_pool(name="dense", bufs=6) as d_pool,
            tc.tile_pool(name="trash", bufs=2) as trash_pool,
            tc.tile_pool(name="scores", bufs=1) as s_pool,
        ):
            scores = s_pool.tile([P, total_slots], f32)

            # --- positives: both sides dense ---
            slot = 0
            left = pos_slots
            col = 0
            while left > 0:
                n = min(left, BATCH_SLOTS)
                A = d_pool.tile([P, BATCH_SLOTS * D], f32, tag="pa")
                nc.sync.dma_start(A[:, 0:n * D],
                                  pos_a_d[:, col * D:(col + n) * D])
                B = d_pool.tile([P, BATCH_SLOTS * D], f32, tag="pb")
                nc.sync.dma_start(B[:, 0:n * D],
                                  pos_b_d[:, col * D:(col + n) * D])
                for s in range(n):
                    tr = trash_pool.tile([P, D], f32, tag="tr")
                    nc.vector.scalar_tensor_tensor(
                        out=tr[:], in0=A[:, s * D:(s + 1) * D], scalar=1.0,
                        in1=B[:, s * D:(s + 1) * D], op0=mult, op1=mult,
                        accum_out=scores[:, slot + s:slot + s + 1])
                col += n
                left -= n
                slot += n

            # --- head / tail: gather + dense ---
            # slot offsets: head section starts at pos_slots, tail after
            sec_base = {id(hidx_d): pos_slots, id(tidx_d): pos_slots + nh}
            for bi, (table, c, idx_d, dense_d, col, n) in enumerate(
                    gather_batches):
                q = bi % 4
                nidx = n * P
                cols = n * 8
                base = sec_base[id(idx_d)] + col
                ia = idx_pool.tile([P, BATCH_SLOTS * 8], i16, tag="ia")
                nc.sync.dma_start(ia[:, 0:cols],
                                  idx_d[:, col * 8:col * 8 + cols])
                G = g_pool.tile([P, BATCH_SLOTS * D], f32, tag="G")
                nc.gpsimd.dma_gather(
                    out_ap=G[:, 0:n * D].rearrange("p (g d) -> p g d", d=D),
                    in_ap=table[c * CHUNK:min((c + 1) * CHUNK, N_A), :],
                    idxs_ap=ia[:, 0:cols],
                    num_idxs=nidx, num_idxs_reg=nidx, elem_size=D,
                    queue_num=q)
                Dn = d_pool.tile([P, BATCH_SLOTS * D], f32, tag="dn")
                nc.sync.dma_start(Dn[:, 0:n * D],
                                  dense_d[:, col * D:(col + n) * D])
                for s in range(n):
                    tr = trash_pool.tile([P, D], f32, tag="tr")
                    nc.vector.scalar_tensor_tensor(
                        out=tr[:], in0=G[:, s * D:(s + 1) * D], scalar=1.0,
                        in1=Dn[:, s * D:(s + 1) * D], op0=mult, op1=mult,
                        accum_out=scores[:, base + s:base + s + 1])

            nc.sync.dma_start(s_out[:], scores[:])

    nc.compile()
    return nc


def _wrap_idx_batched(flat_idx, group_slots):
    """[S, P] int16 per-slot indices -> [P, S*8] dma_gather layout. Batch
    boundaries mirror the device program: per chunk-group, batches of up to
    BATCH_SLOTS slots; each batch's n*128 indices are 16-wrapped and
    replicated across the 8 Q7 cores."""
    S = flat_idx.shape[0]
    assert S == sum(group_slots)
    out = np.empty((P, S * 8), dtype=np.int16)
    col = 0
    s = 0
    for gs in group_slots:
        left = gs
        while left > 0:
            n = min(left, BATCH_SLOTS)
            flat = flat_idx[s:s + n].reshape(-1)       # slot-major, 128 fast
            w16 = flat.reshape(n * P // 16, 16).T      # [16, n*8]
            out[:, col:col + n * 8] = np.tile(w16, (8, 1))
            col += n * 8
            s += n
            left -= n
    return out


def _deal(padded_len, arrs):
    """Reshape [padded_len]-arrays to [slots, NCORES, P] dealt layout."""
    return [a.reshape(-1, NCORES, P) for a in arrs]


def kernel(emb_A, emb_B, rel_kernel, edge_pos, head_batch, tail_batch):
    from concourse.bass_utils import run_bass_kernel_spmd

    emb_A = np.ascontiguousarray(np.asarray(emb_A, dtype=np.float32))
    emb_B = np.ascontiguousarray(np.asarray(emb_B, dtype=np.float32))
    kv = np.asarray(rel_kernel, dtype=np.float32)[0]
    ep = np.asarray(edge_pos, dtype=np.int64)
    hb = np.asarray(head_batch, dtype=np.int64)
    tb = np.asarray(tail_batch, dtype=np.int64)

    # host-side prescaled row lookups (built lazily per needed rows)
    emb_Bk = emb_B * kv[None, :]
    emb_Ak = emb_A * kv[None, :]

    # ---------- positives ----------
    pos_pad = -(-E // SUB) * SUB
    pos_slots = pos_pad // SUB
    a_idx = np.zeros(pos_pad, np.int64)
    b_idx = np.zeros(pos_pad, np.int64)
    outp = np.full(pos_pad, -1, np.int64)
    a_idx[:E], b_idx[:E], outp[:E] = ep[0], ep[1], np.arange(E)
    a_s, b_s, o_s = _deal(pos_pad, [a_idx, b_idx, outp])

    # ---------- head / tail (sorted by corrupt-index chunk) ----------
    def section(corrupt_idx, shared_rows, out_base):
        """corrupt_idx [4E], shared_rows [4E,128] f32 (prescaled side),
        returns (group_slots, per-core idx arrays, dense arrays, outpos)."""
        npair = corrupt_idx.shape[0]
        key = corrupt_idx // CHUNK
        order = np.argsort(key, kind="stable")
        ci_s = corrupt_idx[order]
        op_s = out_base + order
        counts = np.bincount(key, minlength=NCHUNKS)
        group_slots = [int(-(-c // SUB)) for c in counts]
        idx_cores = [[] for _ in range(NCORES)]
        dense_cores = [[] for _ in range(NCORES)]
        outpos_cores = [[] for _ in range(NCORES)]
        start = 0
        for g in range(NCHUNKS):
            cnt = int(counts[g])
            padded = group_slots[g] * SUB
            gi = np.zeros(padded, np.int16)
            gp = np.full(padded, -1, np.int64)
            gi[:cnt] = (ci_s[start:start + cnt] - g * CHUNK).astype(np.int16)
            gp[:cnt] = op_s[start:start + cnt]
            gsh = np.zeros((padded,), np.int64)
            gsh[:cnt] = order[start:start + cnt]
            start += cnt
            gi3, gp3, gsh3 = _deal(padded, [gi, gp, gsh])
            for c in range(NCORES):
                idx_cores[c].append(gi3[:, c, :])
                outpos_cores[c].append(gp3[:, c, :].reshape(-1))
                dense_cores[c].append(gsh3[:, c, :])
        per_core = []
        for c in range(NCORES):
            idx_sp = np.concatenate(idx_cores[c], axis=0)        # [S, P]
            shared_sel = np.concatenate(dense_cores[c], axis=0)  # [S, P]
            dense = shared_rows[shared_sel]                      # [S, P, D]
            dense = np.ascontiguousarray(
                dense.transpose(1, 0, 2).reshape(P, -1))         # [P, S*D]
            per_core.append((
                np.ascontiguousarray(_wrap_idx_batched(idx_sp, group_slots)),
                dense,
                np.concatenate(outpos_cores[c]),
            ))
        return group_slots, per_core

    head_shared = emb_Bk[np.repeat(ep[1], NEG)]     # [4E, D]
    head_slots, head_pc = section(hb.reshape(-1), head_shared, E)
    tail_shared = emb_Ak[np.repeat(ep[0], NEG)]
    tail_slots, tail_pc = section(tb.reshape(-1), tail_shared, 5 * E)

    in_maps = []
    outpos_cores = []
    for c in range(NCORES):
        pos_a = np.ascontiguousarray(
            emb_A[a_s[:, c, :]].transpose(1, 0, 2).reshape(P, -1))
        pos_b = np.ascontiguousarray(
            emb_Bk[b_s[:, c, :]].transpose(1, 0, 2).reshape(P, -1))
        in_maps.append({
            "emb_a": emb_A,
            "emb_b": emb_B,
            "pos_a": pos_a,
            "pos_b": pos_b,
            "head_idx": head_pc[c][0],
            "head_dense": head_pc[c][1],
            "tail_idx": tail_pc[c][0],
            "tail_dense": tail_pc[c][1],
        })
        outpos_cores.append(np.concatenate(
            [o_s[:, c, :].reshape(-1), head_pc[c][2], tail_pc[c][2]]))

    sig = (pos_slots, tuple(head_slots), tuple(tail_slots))
    if _CACHED.get("sig") != sig:
        _CACHED["nc"] = _build_program(pos_slots, head_slots, tail_slots)
        _CACHED["sig"] = sig
    nc = _CACHED["nc"]
    _CACHED["in_maps"] = in_maps
    _CACHED["plan"] = sig

    res = run_bass_kernel_spmd(nc, in_maps, core_ids=list(range(NCORES)))
    _CACHED["last_results"] = res

    out = np.empty(9 * E, dtype=np.float32)
    for c in range(NCORES):
        flat = res.results[c]["scores"].T.reshape(-1)   # j = slot*128 + p
        op = outpos_cores[c]
        valid = op >= 0
        out[op[valid]] = flat[valid]
    return out



# revision 2
# speedup vs baseline: 1.8570x; 1.8570x over previous
"""DistMult edge-scoring kernel for Trainium2 (8 NeuronCores, SPMD).

score[j] = sum_d emb_A[a_idx[j], d] * k[d] * emb_B[b_idx[j], d]
for 9E pairs: E positive edges, 4E head-corrupted, 4E tail-corrupted.

Strategy (v4, fully host-gathered dense bf16 — exploits per-edge row
reuse):
- Per edge e the 9 scores are dots against just 10 rows:
  ak = k*A[ep0[e]], b = B[ep1[e]], hk_j = k*A[head[e,j]],
  t_j = B[tail[e,j]];  pos = ak.b, head_j = hk_j.b, tail_j = ak.t_j.
  The baseline streamed 18 rows/edge; this streams 10.
- ALL rows are gathered on the host (numpy fancy indexing) into one
  dense bf16 array per core, so the device does zero SWDGE gathers
  (the baseline's Q7-descriptor bottleneck) and half the bytes
  (bf16 vs f32; rel_norm error ~2.3e-3, well under the 2e-2 gate).
- Edges are dealt round-robin across the 8 cores in 128-edge blocks;
  every core runs an identical program on 98 slots (12544 edges).
- Per slot: one [128, 1280] bf16 tile (10 rows x 128 d per edge
  partition); 9 fused scalar_tensor_tensor (mul + accumulate-reduce)
  ops produce the 9 score columns. Double-buffered 14-slot DMA
  batches (5.25 MB each) overlap load with compute.
- Host inverse-deals the [128, S*9] score tiles back to reference
  order.
"""

import numpy as np

# problem constants
N_A = 100000
N_B = 100000
D = 128
E = 100000
NEG = 4
NCORES = 8

P = 128
S = 98                 # slots (128-edge blocks) per core: 98*8*128 >= E
BS = 14                # slots per DMA batch (98 = 7*14)
ROW = 10 * D           # free-dim elements per edge: ak|hk0..3|b|t0..3

_CACHED = {}


def _build_program():
    import concourse.tile as tile
    from concourse import bacc, mybir

    f32 = mybir.dt.float32
    bf = mybir.dt.bfloat16
    mult = mybir.AluOpType.mult

    nc = bacc.Bacc("TRN2", target_bir_lowering=False, debug=False,
                   num_devices=NCORES)
    x_d = nc.dram_tensor("x", [P, S * ROW], bf, kind="ExternalInput").ap()
    s_out = nc.dram_tensor("scores", [P, S * 9], f32,
                           kind="ExternalOutput").ap()

    with tile.TileContext(nc) as tc:
        with (
            tc.tile_pool(name="in", bufs=2) as in_pool,
            tc.tile_pool(name="trash", bufs=2) as trash_pool,
            tc.tile_pool(name="sc", bufs=1) as s_pool,
        ):
            scores = s_pool.tile([P, S * 9], f32)
            nb = (S + BS - 1) // BS
            for bi in range(nb):
                n = min(BS, S - bi * BS)
                T = in_pool.tile([P, BS * ROW], bf, tag="x")
                nc.sync.dma_start(
                    T[:, 0:n * ROW],
                    x_d[:, bi * BS * ROW:(bi * BS + n) * ROW])
                for s in range(n):
                    base = (bi * BS + s) * 9
                    o = s * ROW
                    ak = T[:, o:o + D]
                    bc = T[:, o + 5 * D:o + 6 * D]
                    tr = trash_pool.tile([P, D], bf, tag="tr")
                    nc.vector.scalar_tensor_tensor(
                        out=tr[:], in0=ak, scalar=1.0, in1=bc,
                        op0=mult, op1=mult,
                        accum_out=scores[:, base:base + 1])
                    for j in range(4):
                        hk = T[:, o + (1 + j) * D:o + (2 + j) * D]
                        tr = trash_pool.tile([P, D], bf, tag="tr")
                        nc.vector.scalar_tensor_tensor(
                            out=tr[:], in0=hk, scalar=1.0, in1=bc,
                            op0=mult, op1=mult,
                            accum_out=scores[:, base + 1 + j:base + 2 + j])
                    for j in range(4):
                        tj = T[:, o + (6 + j) * D:o + (7 + j) * D]
                        tr = trash_pool.tile([P, D], bf, tag="tr")
                        nc.vector.scalar_tensor_tensor(
                            out=tr[:], in0=tj, scalar=1.0, in1=ak,
                            op0=mult, op1=mult,
                            accum_out=scores[:, base + 5 + j:base + 6 + j])

            nc.sync.dma_start(s_out[:], scores[:])

    nc.compile()
    return nc


def kernel(emb_A, emb_B, rel_kernel, edge_pos, head_batch, tail_batch):
    import ml_dtypes
    from concourse.bass_utils import run_bass_kernel_spmd

    bf16 = ml_dtypes.bfloat16
    emb_A = np.asarray(emb_A, dtype=np.float32)
    emb_B = np.asarray(emb_B, dtype=np.float32)
    kv = np.asarray(rel_kernel, dtype=np.float32)[0]
    ep = np.asarray(edge_pos, dtype=np.int64)
    hb = np.asarray(head_batch, dtype=np.int64)
    tb = np.asarray(tail_batch, dtype=np.int64)

    # prescale k into the A-side table once; round both tables to bf16
    eAk = (emb_A * kv[None, :]).astype(bf16)   # [N_A, D]
    eB16 = emb_B.astype(bf16)                  # [N_B, D]

    # edge e lives at (core c, slot s, partition p): e = (s*8+c)*128 + p
    p_arr = np.arange(P)
    s_arr = np.arange(S)
    in_maps = []
    for c in range(NCORES):
        e = ((s_arr * NCORES + c)[:, None] * P + p_arr[None, :]).ravel()
        esafe = np.where(e < E, e, 0)
        ia = np.empty((S * P, 5), np.int64)
        ia[:, 0] = ep[0][esafe]
        ia[:, 1:] = hb[esafe]
        ib = np.empty((S * P, 5), np.int64)
        ib[:, 0] = ep[1][esafe]
        ib[:, 1:] = tb[esafe]
        x = np.empty((S * P, 10, D), bf16)
        x[:, 0] = eAk[ia[:, 0]]
        x[:, 1:5] = eAk[ia[:, 1:]]
        x[:, 5] = eB16[ib[:, 0]]
        x[:, 6:] = eB16[ib[:, 1:]]
        # [s*P+p, row] -> [p, s*ROW] partition-major device layout
        x = np.ascontiguousarray(
            x.reshape(S, P, ROW).transpose(1, 0, 2).reshape(P, S * ROW))
        in_maps.append({"x": x})

    sig = ("v4", S, BS)
    if _CACHED.get("sig") != sig:
        _CACHED["nc"] = _build_program()
        _CACHED["sig"] = sig
    nc = _CACHED["nc"]
    _CACHED["in_maps"] = in_maps
    _CACHED["plan"] = sig

    res = run_bass_kernel_spmd(nc, in_maps, core_ids=list(range(NCORES)))
    _CACHED["last_results"] = res

    out = np.empty(9 * E, dtype=np.float32)
    for c in range(NCORES):
        sc = res.results[c]["scores"].reshape(P, S, 9)
        e = (s_arr * NCORES + c)[None, :] * P + p_arr[:, None]  # [p, s]
        valid = e < E
        ev = e[valid]
        out[ev] = sc[:, :, 0][valid]
        for j in range(4):
            out[E + ev * 4 + j] = sc[:, :, 1 + j][valid]
            out[5 * E + ev * 4 + j] = sc[:, :, 5 + j][valid]
    return out


# revision 5
# speedup vs baseline: 2.2977x; 1.2373x over previous
"""DistMult edge-scoring kernel for Trainium2 (8 NeuronCores, SPMD).

score[j] = sum_d emb_A[a_idx[j], d] * k[d] * emb_B[b_idx[j], d]
for 9E pairs: E positive edges, 4E head-corrupted, 4E tail-corrupted.

Strategy (v4, fully host-gathered dense bf16 — exploits per-edge row
reuse):
- Per edge e the 9 scores are dots against just 10 rows:
  ak = k*A[ep0[e]], b = B[ep1[e]], hk_j = k*A[head[e,j]],
  t_j = B[tail[e,j]];  pos = ak.b, head_j = hk_j.b, tail_j = ak.t_j.
  The baseline streamed 18 rows/edge; this streams 10.
- ALL rows are gathered on the host (numpy fancy indexing) into one
  dense bf16 array per core, so the device does zero SWDGE gathers
  (the baseline's Q7-descriptor bottleneck) and half the bytes
  (bf16 vs f32; rel_norm error ~2.3e-3, well under the 2e-2 gate).
- Edges are dealt round-robin across the 8 cores in 128-edge blocks;
  every core runs an identical program on 98 slots (12544 edges).
- Per slot: one [128, 1280] bf16 tile (10 rows x 128 d per edge
  partition); 9 fused scalar_tensor_tensor (mul + accumulate-reduce)
  ops produce the 9 score columns. Double-buffered 14-slot DMA
  batches (5.25 MB each) overlap load with compute.
- Host inverse-deals the [128, S*9] score tiles back to reference
  order.
"""

import numpy as np

# problem constants
N_A = 100000
N_B = 100000
D = 128
E = 100000
NEG = 4
NCORES = 8

P = 128
S = 98                 # slots (128-edge blocks) per core: 98*8*128 >= E
BS = 14                # slots per DMA batch (98 = 7*14)
ROW = 10 * D           # free-dim elements per edge: ak|hk0..3|b|t0..3

_CACHED = {}


def _build_program():
    import concourse.tile as tile
    from concourse import bacc, mybir

    f32 = mybir.dt.float32
    bf = mybir.dt.bfloat16
    mult = mybir.AluOpType.mult

    nc = bacc.Bacc("TRN2", target_bir_lowering=False, debug=False,
                   num_devices=NCORES)
    x_d = nc.dram_tensor("x", [P, S * ROW], bf, kind="ExternalInput").ap()
    s_out = nc.dram_tensor("scores", [P, S * 9], f32,
                           kind="ExternalOutput").ap()

    with tile.TileContext(nc) as tc:
        with (
            tc.tile_pool(name="in", bufs=2) as in_pool,
            tc.tile_pool(name="trash", bufs=1) as trash_pool,
            tc.tile_pool(name="sc", bufs=1) as s_pool,
        ):
            # col layout: [0, S*5) = per-slot [pos, head0..3];
            #             [S*5, S*9) = per-slot [tail0..3]
            scores = s_pool.tile([P, S * 9], f32)
            nb = (S + BS - 1) // BS
            for bi in range(nb):
                n = min(BS, S - bi * BS)
                T = in_pool.tile([P, BS * ROW], bf, tag="x")
                nc.sync.dma_start(
                    T[:, 0:n * ROW],
                    x_d[:, bi * BS * ROW:(bi * BS + n) * ROW])
                R = T[:, 0:n * ROW].rearrange("p (s t d) -> p s t d",
                                              t=10, d=D)
                lhs1 = R[:, :, 0:5, :]                       # ak|hk0..3
                b_b = R[:, :, 5:6, :].broadcast_to([P, n, 5, D])
                lhs2 = R[:, :, 6:10, :]                      # t0..3
                ak_b = R[:, :, 0:1, :].broadcast_to([P, n, 4, D])
                pr1 = trash_pool.tile([P, BS * 5 * D], bf, tag="p1")
                pr2 = trash_pool.tile([P, BS * 4 * D], bf, tag="p2")
                p1 = pr1[:, 0:n * 5 * D].rearrange(
                    "p (s t d) -> p s t d", t=5, d=D)
                p2 = pr2[:, 0:n * 4 * D].rearrange(
                    "p (s t d) -> p s t d", t=4, d=D)
                nc.vector.tensor_tensor(out=p1, in0=lhs1, in1=b_b, op=mult)
                nc.vector.tensor_tensor(out=p2, in0=lhs2, in1=ak_b, op=mult)
                nc.vector.reduce_sum(
                    out=scores[:, bi * BS * 5:(bi * BS + n) * 5],
                    in_=pr1[:, 0:n * 5 * D].rearrange(
                        "p (c d) -> p c d", d=D),
                    axis=mybir.AxisListType.X)
                nc.vector.reduce_sum(
                    out=scores[:, S * 5 + bi * BS * 4:S * 5 + (bi * BS + n) * 4],
                    in_=pr2[:, 0:n * 4 * D].rearrange(
                        "p (c d) -> p c d", d=D),
                    axis=mybir.AxisListType.X)

            nc.sync.dma_start(s_out[:], scores[:])

    nc.compile()
    return nc


def kernel(emb_A, emb_B, rel_kernel, edge_pos, head_batch, tail_batch):
    import ml_dtypes
    from concourse.bass_utils import run_bass_kernel_spmd

    bf16 = ml_dtypes.bfloat16
    emb_A = np.asarray(emb_A, dtype=np.float32)
    emb_B = np.asarray(emb_B, dtype=np.float32)
    kv = np.asarray(rel_kernel, dtype=np.float32)[0]
    ep = np.asarray(edge_pos, dtype=np.int64)
    hb = np.asarray(head_batch, dtype=np.int64)
    tb = np.asarray(tail_batch, dtype=np.int64)

    # prescale k into the A-side table once; round both tables to bf16
    eAk = (emb_A * kv[None, :]).astype(bf16)   # [N_A, D]
    eB16 = emb_B.astype(bf16)                  # [N_B, D]

    # edge e lives at (core c, slot s, partition p): e = (s*8+c)*128 + p
    p_arr = np.arange(P)
    s_arr = np.arange(S)
    in_maps = []
    for c in range(NCORES):
        e = ((s_arr * NCORES + c)[:, None] * P + p_arr[None, :]).ravel()
        esafe = np.where(e < E, e, 0)
        ia = np.empty((S * P, 5), np.int64)
        ia[:, 0] = ep[0][esafe]
        ia[:, 1:] = hb[esafe]
        ib = np.empty((S * P, 5), np.int64)
        ib[:, 0] = ep[1][esafe]
        ib[:, 1:] = tb[esafe]
        x = np.empty((S * P, 10, D), bf16)
        x[:, 0] = eAk[ia[:, 0]]
        x[:, 1:5] = eAk[ia[:, 1:]]
        x[:, 5] = eB16[ib[:, 0]]
        x[:, 6:] = eB16[ib[:, 1:]]
        # [s*P+p, row] -> [p, s*ROW] partition-major device layout
        x = np.ascontiguousarray(
            x.reshape(S, P, ROW).transpose(1, 0, 2).reshape(P, S * ROW))
        in_maps.append({"x": x})

    sig = ("v5", S, BS)
    if _CACHED.get("sig") != sig:
        _CACHED["nc"] = _build_program()
        _CACHED["sig"] = sig
    nc = _CACHED["nc"]
    _CACHED["in_maps"] = in_maps
    _CACHED["plan"] = sig

    res = run_bass_kernel_spmd(nc, in_maps, core_ids=list(range(NCORES)))
    _CACHED["last_results"] = res

    out = np.empty(9 * E, dtype=np.float32)
    for c in range(NCORES):
        flat = res.results[c]["scores"]
        sc1 = flat[:, :S * 5].reshape(P, S, 5)   # pos|head0..3
        sc2 = flat[:, S * 5:].reshape(P, S, 4)   # tail0..3
        e = (s_arr * NCORES + c)[None, :] * P + p_arr[:, None]  # [p, s]
        valid = e < E
        ev = e[valid]
        out[ev] = sc1[:, :, 0][valid]
        for j in range(4):
            out[E + ev * 4 + j] = sc1[:, :, 1 + j][valid]
            out[5 * E + ev * 4 + j] = sc2[:, :, j][valid]
    return out


# revision 10
# speedup vs baseline: 2.2988x; 1.0005x over previous
"""DistMult edge-scoring kernel for Trainium2 (8 NeuronCores, SPMD).

score[j] = sum_d emb_A[a_idx[j], d] * k[d] * emb_B[b_idx[j], d]
for 9E pairs: E positive edges, 4E head-corrupted, 4E tail-corrupted.

Strategy (v4, fully host-gathered dense bf16 — exploits per-edge row
reuse):
- Per edge e the 9 scores are dots against just 10 rows:
  ak = k*A[ep0[e]], b = B[ep1[e]], hk_j = k*A[head[e,j]],
  t_j = B[tail[e,j]];  pos = ak.b, head_j = hk_j.b, tail_j = ak.t_j.
  The baseline streamed 18 rows/edge; this streams 10.
- ALL rows are gathered on the host (numpy fancy indexing) into one
  dense bf16 array per core, so the device does zero SWDGE gathers
  (the baseline's Q7-descriptor bottleneck) and half the bytes
  (bf16 vs f32; rel_norm error ~2.3e-3, well under the 2e-2 gate).
- Edges are dealt round-robin across the 8 cores in 128-edge blocks;
  every core runs an identical program on 98 slots (12544 edges).
- Per slot: one [128, 1280] bf16 tile (10 rows x 128 d per edge
  partition); 9 fused scalar_tensor_tensor (mul + accumulate-reduce)
  ops produce the 9 score columns. Double-buffered 14-slot DMA
  batches (5.25 MB each) overlap load with compute.
- Host inverse-deals the [128, S*9] score tiles back to reference
  order.
"""

import numpy as np

# problem constants
N_A = 100000
N_B = 100000
D = 128
E = 100000
NEG = 4
NCORES = 8

P = 128
S = 98                 # slots (128-edge blocks) per core: 98*8*128 >= E
BS = 14                # slots per DMA batch (98 = 7*14)
ROW = 10 * D           # free-dim elements per edge: ak|hk0..3|b|t0..3

_CACHED = {}


def _build_program():
    import concourse.tile as tile
    from concourse import bacc, mybir

    f32 = mybir.dt.float32
    bf = mybir.dt.bfloat16
    mult = mybir.AluOpType.mult

    nc = bacc.Bacc("TRN2", target_bir_lowering=False, debug=False,
                   num_devices=NCORES)
    x_d = nc.dram_tensor("x", [P, S * ROW], bf, kind="ExternalInput").ap()
    s_out = nc.dram_tensor("scores", [P, S * 9], bf,
                           kind="ExternalOutput").ap()

    with tile.TileContext(nc) as tc:
        with (
            tc.tile_pool(name="in", bufs=2) as in_pool,
            tc.tile_pool(name="trash", bufs=1) as trash_pool,
            tc.tile_pool(name="sc", bufs=1) as s_pool,
        ):
            # col layout: [0, S*5) = per-slot [pos, head0..3];
            #             [S*5, S*9) = per-slot [tail0..3]
            # bf16 scores keep every DVE operand 2-byte so both the
            # multiply and the reduce run in the packed 2x perf mode.
            scores = s_pool.tile([P, S * 9], bf)
            nb = (S + BS - 1) // BS
            for bi in range(nb):
                n = min(BS, S - bi * BS)
                h = n // 2
                T = in_pool.tile([P, BS * ROW], bf, tag="x")
                # split each batch across the two HWDGE rings
                # (sync + scalar) so both generate descriptors
                nc.sync.dma_start(
                    T[:, 0:h * ROW],
                    x_d[:, bi * BS * ROW:(bi * BS + h) * ROW])
                nc.scalar.dma_start(
                    T[:, h * ROW:n * ROW],
                    x_d[:, (bi * BS + h) * ROW:(bi * BS + n) * ROW])
                R = T[:, 0:n * ROW].rearrange("p (s t d) -> p s t d",
                                              t=10, d=D)
                lhs1 = R[:, :, 0:5, :]                       # ak|hk0..3
                b_b = R[:, :, 5:6, :].broadcast_to([P, n, 5, D])
                lhs2 = R[:, :, 6:10, :]                      # t0..3
                ak_b = R[:, :, 0:1, :].broadcast_to([P, n, 4, D])
                pr1 = trash_pool.tile([P, BS * 5 * D], bf, tag="p1")
                pr2 = trash_pool.tile([P, BS * 4 * D], bf, tag="p2")
                p1 = pr1[:, 0:n * 5 * D].rearrange(
                    "p (s t d) -> p s t d", t=5, d=D)
                p2 = pr2[:, 0:n * 4 * D].rearrange(
                    "p (s t d) -> p s t d", t=4, d=D)
                nc.vector.tensor_tensor(out=p1, in0=lhs1, in1=b_b, op=mult)
                nc.vector.tensor_tensor(out=p2, in0=lhs2, in1=ak_b, op=mult)
                with nc.allow_low_precision(
                        reason="bf16 scores; reduce accumulates fp32 "
                               "internally, only the final write rounds"):
                    nc.vector.reduce_sum(
                        out=scores[:, bi * BS * 5:(bi * BS + n) * 5],
                        in_=pr1[:, 0:n * 5 * D].rearrange(
                            "p (c d) -> p c d", d=D),
                        axis=mybir.AxisListType.X)
                    nc.vector.reduce_sum(
                        out=scores[:, S * 5 + bi * BS * 4:
                                   S * 5 + (bi * BS + n) * 4],
                        in_=pr2[:, 0:n * 4 * D].rearrange(
                            "p (c d) -> p c d", d=D),
                        axis=mybir.AxisListType.X)

            nc.sync.dma_start(s_out[:], scores[:])

    nc.compile()
    return nc


def kernel(emb_A, emb_B, rel_kernel, edge_pos, head_batch, tail_batch):
    import ml_dtypes
    from concourse.bass_utils import run_bass_kernel_spmd

    bf16 = ml_dtypes.bfloat16
    emb_A = np.asarray(emb_A, dtype=np.float32)
    emb_B = np.asarray(emb_B, dtype=np.float32)
    kv = np.asarray(rel_kernel, dtype=np.float32)[0]
    ep = np.asarray(edge_pos, dtype=np.int64)
    hb = np.asarray(head_batch, dtype=np.int64)
    tb = np.asarray(tail_batch, dtype=np.int64)

    # prescale k into the A-side table once; round both tables to bf16
    eAk = (emb_A * kv[None, :]).astype(bf16)   # [N_A, D]
    eB16 = emb_B.astype(bf16)                  # [N_B, D]

    # edge e lives at (core c, slot s, partition p): e = (s*8+c)*128 + p
    p_arr = np.arange(P)
    s_arr = np.arange(S)
    in_maps = []
    for c in range(NCORES):
        e = ((s_arr * NCORES + c)[:, None] * P + p_arr[None, :]).ravel()
        esafe = np.where(e < E, e, 0)
        ia = np.empty((S * P, 5), np.int64)
        ia[:, 0] = ep[0][esafe]
        ia[:, 1:] = hb[esafe]
        ib = np.empty((S * P, 5), np.int64)
        ib[:, 0] = ep[1][esafe]
        ib[:, 1:] = tb[esafe]
        x = np.empty((S * P, 10, D), bf16)
        x[:, 0] = eAk[ia[:, 0]]
        x[:, 1:5] = eAk[ia[:, 1:]]
        x[:, 5] = eB16[ib[:, 0]]
        x[:, 6:] = eB16[ib[:, 1:]]
        # [s*P+p, row] -> [p, s*ROW] partition-major device layout
        x = np.ascontiguousarray(
            x.reshape(S, P, ROW).transpose(1, 0, 2).reshape(P, S * ROW))
        in_maps.append({"x": x})

    sig = ("v6", S, BS)
    if _CACHED.get("sig") != sig:
        _CACHED["nc"] = _build_program()
        _CACHED["sig"] = sig
    nc = _CACHED["nc"]
    _CACHED["in_maps"] = in_maps
    _CACHED["plan"] = sig

    res = run_bass_kernel_spmd(nc, in_maps, core_ids=list(range(NCORES)))
    _CACHED["last_results"] = res

    out = np.empty(9 * E, dtype=np.float32)
    for c in range(NCORES):
        flat = res.results[c]["scores"].astype(np.float32)
        sc1 = flat[:, :S * 5].reshape(P, S, 5)   # pos|head0..3
        sc2 = flat[:, S * 5:].reshape(P, S, 4)   # tail0..3
        e = (s_arr * NCORES + c)[None, :] * P + p_arr[:, None]  # [p, s]
        valid = e < E
        ev = e[valid]
        out[ev] = sc1[:, :, 0][valid]
        for j in range(4):
            out[E + ev * 4 + j] = sc1[:, :, 1 + j][valid]
            out[5 * E + ev * 4 + j] = sc2[:, :, j][valid]
    return out


# revision 16
# speedup vs baseline: 3.0781x; 1.3390x over previous
"""DistMult edge-scoring kernel for Trainium2 (8 NeuronCores, SPMD).

score[j] = sum_d emb_A[a_idx[j], d] * k[d] * emb_B[b_idx[j], d]
for 9E pairs: E positive edges, 4E head-corrupted, 4E tail-corrupted.

Strategy (v4, fully host-gathered dense bf16 — exploits per-edge row
reuse):
- Per edge e the 9 scores are dots against just 10 rows:
  ak = k*A[ep0[e]], b = B[ep1[e]], hk_j = k*A[head[e,j]],
  t_j = B[tail[e,j]];  pos = ak.b, head_j = hk_j.b, tail_j = ak.t_j.
  The baseline streamed 18 rows/edge; this streams 10.
- ALL rows are gathered on the host (numpy fancy indexing) into one
  dense bf16 array per core, so the device does zero SWDGE gathers
  (the baseline's Q7-descriptor bottleneck) and half the bytes
  (bf16 vs f32; rel_norm error ~2.3e-3, well under the 2e-2 gate).
- Edges are dealt round-robin across the 8 cores in 128-edge blocks;
  every core runs an identical program on 98 slots (12544 edges).
- Per slot: one [128, 1280] bf16 tile (10 rows x 128 d per edge
  partition); 9 fused scalar_tensor_tensor (mul + accumulate-reduce)
  ops produce the 9 score columns. Double-buffered 14-slot DMA
  batches (5.25 MB each) overlap load with compute.
- Host inverse-deals the [128, S*9] score tiles back to reference
  order.
"""

import numpy as np

# problem constants
N_A = 100000
N_B = 100000
D = 128
E = 100000
NEG = 4
NCORES = 8

P = 128
S = 98                 # slots (128-edge blocks) per core: 98*8*128 >= E
BS = 14                # slots per DMA batch (98 = 7*14)
ROW = 10 * D           # free-dim elements per edge: ak|hk0..3|b|t0..3

_CACHED = {}


def _build_program():
    import concourse.tile as tile
    from concourse import bacc, mybir

    f32 = mybir.dt.float32  # noqa: F841
    bf = mybir.dt.bfloat16
    mult = mybir.AluOpType.mult
    add = mybir.AluOpType.add

    nc = bacc.Bacc("TRN2", target_bir_lowering=False, debug=False,
                   num_devices=NCORES)
    x_d = nc.dram_tensor("x", [P, S * ROW], bf, kind="ExternalInput").ap()
    s_out = nc.dram_tensor("scores", [P, S * 9], bf,
                           kind="ExternalOutput").ap()

    with tile.TileContext(nc) as tc:
        with (
            tc.tile_pool(name="in", bufs=2) as in_pool,
            tc.tile_pool(name="trash", bufs=1) as trash_pool,
            tc.tile_pool(name="sc", bufs=1) as s_pool,
        ):
            # scores col layout: per batch bi a block of n*9 columns:
            # first n*5 = (slot-local s, [pos, head0..3]),
            # then  n*4 = (slot-local s, [tail0..3]).
            # bf16 everywhere keeps every DVE operand 2-byte so the
            # multiply AND the whole add-tree run in packed 2x mode
            # (TENSOR_REDUCE has no 2x uop -> replaced by a TT-add tree).
            scores = s_pool.tile([P, S * 9], bf)
            nb = (S + BS - 1) // BS
            for bi in range(nb):
                n = min(BS, S - bi * BS)
                h = n // 2
                C = n * 9
                T = in_pool.tile([P, BS * ROW], bf, tag="x")
                # split each batch across the two HWDGE rings
                # (sync + scalar) so both generate descriptors
                nc.sync.dma_start(
                    T[:, 0:h * ROW],
                    x_d[:, bi * BS * ROW:(bi * BS + h) * ROW])
                nc.scalar.dma_start(
                    T[:, h * ROW:n * ROW],
                    x_d[:, (bi * BS + h) * ROW:(bi * BS + n) * ROW])
                R = T[:, 0:n * ROW].rearrange("p (s t d) -> p s t d",
                                              t=10, d=D)
                lhs1 = R[:, :, 0:5, :]                       # ak|hk0..3
                b_b = R[:, :, 5:6, :].broadcast_to([P, n, 5, D])
                lhs2 = R[:, :, 6:10, :]                      # t0..3
                ak_b = R[:, :, 0:1, :].broadcast_to([P, n, 4, D])
                pr = trash_pool.tile([P, BS * 9 * D], bf, tag="pr")
                p1 = pr[:, 0:n * 5 * D].rearrange(
                    "p (s t d) -> p s t d", t=5, d=D)
                p2 = pr[:, n * 5 * D:n * 9 * D].rearrange(
                    "p (s t d) -> p s t d", t=4, d=D)
                nc.vector.tensor_tensor(out=p1, in0=lhs1, in1=b_b, op=mult)
                nc.vector.tensor_tensor(out=p2, in0=lhs2, in1=ak_b, op=mult)
                # binary add-tree over d (2x packed TT adds), stopped at
                # width 4 to keep every operand 4B-aligned with innermost
                # count >= 2; a final cheap 1x reduce finishes 4 -> 1.
                cur = pr[:, 0:C * D].rearrange("p (c d) -> p c d", d=D)
                w = D
                while w > 4:
                    hw_ = w // 2
                    nxt_t = trash_pool.tile([P, BS * 9 * hw_], bf,
                                            tag=f"h{hw_}")
                    nxt = nxt_t[:, 0:C * hw_].rearrange(
                        "p (c d) -> p c d", d=hw_)
                    nc.vector.tensor_tensor(
                        out=nxt, in0=cur[:, :, 0:hw_],
                        in1=cur[:, :, hw_:w], op=add)
                    cur = nxt
                    w = hw_
                with nc.allow_low_precision(
                        reason="bf16 scores; reduce accumulates fp32 "
                               "internally, only the final write rounds"):
                    nc.vector.reduce_sum(
                        out=scores[:, bi * BS * 9:bi * BS * 9 + C],
                        in_=cur, axis=mybir.AxisListType.X)

            nc.sync.dma_start(s_out[:], scores[:])

    nc.compile()
    return nc


def kernel(emb_A, emb_B, rel_kernel, edge_pos, head_batch, tail_batch):
    import ml_dtypes
    from concourse.bass_utils import run_bass_kernel_spmd

    bf16 = ml_dtypes.bfloat16
    emb_A = np.asarray(emb_A, dtype=np.float32)
    emb_B = np.asarray(emb_B, dtype=np.float32)
    kv = np.asarray(rel_kernel, dtype=np.float32)[0]
    ep = np.asarray(edge_pos, dtype=np.int64)
    hb = np.asarray(head_batch, dtype=np.int64)
    tb = np.asarray(tail_batch, dtype=np.int64)

    # prescale k into the A-side table once; round both tables to bf16
    eAk = (emb_A * kv[None, :]).astype(bf16)   # [N_A, D]
    eB16 = emb_B.astype(bf16)                  # [N_B, D]

    # edge e lives at (core c, slot s, partition p): e = (s*8+c)*128 + p
    p_arr = np.arange(P)
    s_arr = np.arange(S)
    in_maps = []
    for c in range(NCORES):
        e = ((s_arr * NCORES + c)[:, None] * P + p_arr[None, :]).ravel()
        esafe = np.where(e < E, e, 0)
        ia = np.empty((S * P, 5), np.int64)
        ia[:, 0] = ep[0][esafe]
        ia[:, 1:] = hb[esafe]
        ib = np.empty((S * P, 5), np.int64)
        ib[:, 0] = ep[1][esafe]
        ib[:, 1:] = tb[esafe]
        x = np.empty((S * P, 10, D), bf16)
        x[:, 0] = eAk[ia[:, 0]]
        x[:, 1:5] = eAk[ia[:, 1:]]
        x[:, 5] = eB16[ib[:, 0]]
        x[:, 6:] = eB16[ib[:, 1:]]
        # [s*P+p, row] -> [p, s*ROW] partition-major device layout
        x = np.ascontiguousarray(
            x.reshape(S, P, ROW).transpose(1, 0, 2).reshape(P, S * ROW))
        in_maps.append({"x": x})

    sig = ("v7b", S, BS)
    if _CACHED.get("sig") != sig:
        _CACHED["nc"] = _build_program()
        _CACHED["sig"] = sig
    nc = _CACHED["nc"]
    _CACHED["in_maps"] = in_maps
    _CACHED["plan"] = sig

    res = run_bass_kernel_spmd(nc, in_maps, core_ids=list(range(NCORES)))
    _CACHED["last_results"] = res

    out = np.empty(9 * E, dtype=np.float32)
    for c in range(NCORES):
        flat = res.results[c]["scores"].astype(np.float32)
        blk = flat.reshape(P, S // BS, BS * 9)
        sc1 = blk[:, :, :BS * 5].reshape(P, S, 5)   # pos|head0..3
        sc2 = blk[:, :, BS * 5:].reshape(P, S, 4)   # tail0..3
        e = (s_arr * NCORES + c)[None, :] * P + p_arr[:, None]  # [p, s]
        valid = e < E
        ev = e[valid]
        out[ev] = sc1[:, :, 0][valid]
        for j in range(4):
            out[E + ev * 4 + j] = sc1[:, :, 1 + j][valid]
            out[5 * E + ev * 4 + j] = sc2[:, :, j][valid]
    return out
